# revision 17
# baseline (speedup 1.0000x reference)
"""Multi-head attention (B=2, S=2048, D=1024, H=16, Hd=64) on 8 trn2 cores.

Sharding: batch x head-group. Core c handles batch c//4 and heads
[4*(c%4), 4*(c%4)+4).

Numerics (validated in numpy against the fp32 reference, rel ~4e-3):
- Q/K projections: fp8e4m3 x and fp8 weights (pre-scaled by 32 to avoid
  the fp8 subnormal region), DoubleRow perf mode: each matmul contracts
  256 rows at 0.5 cycles/output-column.
- V projection: compensated fp8 (x8*Wv8 + x8*rv8 + r8*Wv8 where r8/rv8
  are fp8 residuals) - plain fp8 V fails the 2e-2 gate, fp16 costs 2x.
- Scores: fp16 Q/K operands (64-dim contraction per head).
- Softmax: exp on ScalarE with scale 1/(32*32*32) (weight prescale
  folded in); no max-subtraction needed (|score| < ~0.4).
- PV: flipped layout - P^T tiles become the stationary operand so the
  output lands as [query-partitions, head-dim]; the ones-column of V
  gives the softmax row-sum per query partition, so normalization is a
  per-partition reciprocal + tensor_scalar multiply (no DRAM bounce).
- Attention outputs are transposed back to [dl, q] with PE transposes
  (identity matmul) for the fp16 output projection; partial outputs are
  stored fp16 and summed on the host (bv @ Wo.T + bo added analytically).

Mask: reference keeps the *upper* triangle (key >= query): query q
attends keys k >= q; tiles strictly below the block diagonal are skipped.
Key-chunk pairs are processed ascending so V tiles and KT chunks are
consumed in DMA-arrival order; diagonal pairs pack the second key
chunk's scores adjacent to the first so one exp covers both.

PSUM budget (8 banks): scores 2x[128,1024] (4) + PV accumulators
[128,1024] x1 (2) + outproj/transpose shared pool (2). PV packs 8
(head, q-chunk) accumulators of 65 fp32 into 2 banks; a cheap
zero-outer-product matmul pre-zeroes each bank so the packed slots can
all accumulate with start=False (PSUM start=True zeroing is
2KB-region-granular).
"""

import contextlib

import os as _os

_jp = _os.environ.get("JAX_PLATFORMS", "")
if _jp and "axon" not in _jp:
    _os.environ["JAX_PLATFORMS"] = "axon," + _jp

import numpy as np
import ml_dtypes

import concourse.bass as bass
import concourse.tile as tile
from concourse import bacc, mybir
from concourse.bass_utils import run_bass_kernel_spmd

F32 = mybir.dt.float32
F16 = mybir.dt.float16
F8 = mybir.dt.float8e4
NP8 = ml_dtypes.float8_e4m3

B = 2
S = 2048
D = 1024
HD = 64
N_CORES = 8
HPC = 4  # heads per core
DSL = HPC * HD  # 256 projection columns per core
P = 128
NST = S // P  # 16 seq tiles
QCH = 512
NQC = S // QCH  # 4

WS = 32.0  # weight prescale (keeps fp8 weights out of the subnormal range)
ESCALE = 1.0 / (np.sqrt(np.float32(D)) * WS * WS)  # exp scale: 1/32768

DR = mybir.MatmulPerfMode.DoubleRow
EXP = mybir.ActivationFunctionType.Exp


def _build_kernel(nc: bass.Bass, repeat: int = 1):
    x8d = nc.dram_tensor("x8p", (P, 4, 2, S), F8, kind="ExternalInput").ap()
    xr8d = nc.dram_tensor("xr8p", (P, 4, 2, S), F8, kind="ExternalInput").ap()
    w8d = nc.dram_tensor("w8p", (P, 4, 2, 3 * DSL), F8, kind="ExternalInput").ap()
    wvr8d = nc.dram_tensor("wvr8p", (P, 4, 2, DSL), F8, kind="ExternalInput").ap()
    wod = nc.dram_tensor("woT", (2, P, D), F16, kind="ExternalInput").ap()
    bqkd = nc.dram_tensor("bqk", (P, 2, 2), F32, kind="ExternalInput").ap()
    identd = nc.dram_tensor("ident", (P, P), F16, kind="ExternalInput").ap()
    outp = nc.dram_tensor("outp", (S, D), F16, kind="ExternalOutput").ap()

    with tile.TileContext(nc) as tc:
        for _ in range(repeat):
            _emit(tc, nc, x8d, xr8d, w8d, wvr8d, wod, bqkd, identd, outp)
    nc.compile()
    return nc


def _emit(tc, nc, x8d, xr8d, w8d, wvr8d, wod, bqkd, identd, outp):
    ctx = contextlib.ExitStack()

    persist = ctx.enter_context(tc.tile_pool(name="persist", bufs=1))

    x8_t = persist.tile([P, 4, 2, S], F8, tag="x8", name="x8")
    xr8_t = persist.tile([P, 4, 2, S], F8, tag="xr8", name="xr8")
    w8_t = persist.tile([P, 4, 2, 3 * DSL], F8, tag="w8", name="w8")
    wvr8_t = persist.tile([P, 4, 2, DSL], F8, tag="wvr", name="wvr")
    wo_t = [persist.tile([P, D], F16, tag=f"wo{j}", name=f"wo{j}") for j in range(2)]
    bias_sb = persist.tile([P, 2, 2], F32, tag="bias", name="bias")
    ident_sb = persist.tile([P, P], F16, tag="ident", name="ident")
    zeros_sb = persist.tile([P, QCH], F16, tag="zeros", name="zeros")
    qt_sb = [persist.tile([P, S], F8, tag=f"qt{j}", name=f"qt{j}") for j in range(2)]
    kt_sb = [persist.tile([P, S], F8, tag=f"kt{j}", name=f"kt{j}") for j in range(2)]
    q8_sb = [persist.tile([P, 2, S], F8, tag=f"q8{j}", name=f"q8{j}") for j in range(2)]
    k8_sb = [persist.tile([P, 2, S], F8, tag=f"k8{j}", name=f"k8{j}") for j in range(2)]
    v_sb = [
        persist.tile([P, HPC, HD + 1], F16, tag=f"v{i}", name=f"v{i}")
        for i in range(NST)
    ]
    attn_sb = [
        persist.tile([P, NST, P], F16, tag=f"attn{j}", name=f"attn{j}")
        for j in range(2)
    ]
    attnt_sb = [
        persist.tile([P, S], F16, tag=f"attnt{j}", name=f"attnt{j}") for j in range(2)
    ]
    rinv_sb = [
        persist.tile([P, NST, 2], F32, tag=f"rinv{j}", name=f"rinv{j}")
        for j in range(2)
    ]

    nc.vector.memset(zeros_sb[:], 0.0)

    # --- input DMAs (SP queue; s-quartered so early seq chunks arrive first;
    # Q/K weight columns before V so the first score tile is unblocked ASAP)
    nc.sync.dma_start(out=bias_sb[:], in_=bqkd)
    nc.sync.dma_start(out=x8_t[:, :, :, 0:QCH], in_=x8d[:, :, :, 0:QCH])
    nc.sync.dma_start(out=w8_t[:, :, :, 0 : 2 * DSL], in_=w8d[:, :, :, 0 : 2 * DSL])
    nc.sync.dma_start(out=xr8_t[:, :, :, 0:QCH], in_=xr8d[:, :, :, 0:QCH])

    def early_loads2():
        nc.sync.dma_start(
            out=w8_t[:, :, :, 2 * DSL : 3 * DSL], in_=w8d[:, :, :, 2 * DSL : 3 * DSL]
        )
        nc.sync.dma_start(out=wvr8_t[:], in_=wvr8d)
        nc.sync.dma_start(out=ident_sb[:], in_=identd)
        nc.sync.dma_start(
            out=x8_t[:, :, :, QCH : 2 * QCH], in_=x8d[:, :, :, QCH : 2 * QCH]
        )
        nc.sync.dma_start(
            out=xr8_t[:, :, :, QCH : 2 * QCH], in_=xr8d[:, :, :, QCH : 2 * QCH]
        )

    def late_loads(q4):
        if q4 < 4:
            ssl = slice(q4 * QCH, (q4 + 1) * QCH)
            nc.sync.dma_start(out=x8_t[:, :, :, ssl], in_=x8d[:, :, :, ssl])
            nc.sync.dma_start(out=xr8_t[:, :, :, ssl], in_=xr8d[:, :, :, ssl])
        else:
            for j in range(2):
                nc.sync.dma_start(out=wo_t[j][:], in_=wod[j])

    st_pool = ctx.enter_context(tc.tile_pool(name="st_psum", bufs=2, space="PSUM"))
    pv_pool = ctx.enter_context(tc.tile_pool(name="pv_psum", bufs=1, space="PSUM"))
    pt_pool = ctx.enter_context(tc.tile_pool(name="pt", bufs=6))
    ob_pool = ctx.enter_context(tc.tile_pool(name="ob", bufs=4))
    op_psum_cell = []

    # PV accumulator slot: m = h*4 + ql; slots 0-6 packed in bank 0,
    # slot 7 at the start of bank 1 (matmul outputs may not straddle banks).
    def pv_slot(pv, h, ql):
        m = h * 4 + ql
        if m < 7:
            return pv[:, 65 * m : 65 * m + 65]
        return pv[:, 512 : 512 + 65]

    def _transp(hp, qq):
        tp = op_psum_cell[0].tile([P, P], F16, tag="op", name="tp")
        nc.tensor.transpose(tp[:], attn_sb[hp][:, qq, :], ident_sb[:])
        if hp == 1 and qq >= 12:
            nc.scalar.copy(attnt_sb[hp][:, qq * P : (qq + 1) * P], tp[:])
        else:
            nc.vector.tensor_copy(attnt_sb[hp][:, qq * P : (qq + 1) * P], tp[:])

    ob_tiles = {}

    def outproj_unit(sti, e, slotkind="op"):
        if e == 0:
            ob_tiles[sti] = ob_pool.tile([P, D], F16, tag="ob", name="ob")
        ob = ob_tiles[sti]
        if slotkind == "st":
            # scores are done: borrow a free st-pool bank for extra slots
            stt = st_pool.tile([P, 1024], F32, tag="st", name="sttail")
            op = stt[:, e * QCH : (e + 1) * QCH]
        elif slotkind == "pv":
            pvt = pv_pool.tile([P, 1024], F32, tag="pv", name="pvtail")
            op = pvt[:, e * QCH : (e + 1) * QCH]
        else:
            op = op_psum_cell[0].tile([P, QCH], F32, tag="op", name="op")[:]
        for j in range(2):
            nc.tensor.matmul(
                op,
                lhsT=attnt_sb[j][:, sti * P : (sti + 1) * P],
                rhs=wo_t[j][:, e * QCH : (e + 1) * QCH],
                start=(j == 0),
                stop=(j == 1),
                skip_group_check=(slotkind == "pv"),
            )
        esl = slice(e * QCH, (e + 1) * QCH)
        # GPSIMD cannot read PSUM: evictions go to DVE; late ones to the
        # mostly-idle ScalarE
        if sti >= 8 and e == 0:
            nc.scalar.copy(ob[:, esl], op)
        else:
            nc.vector.tensor_copy(ob[:, esl], op)
        if e == 1:
            nc.sync.dma_start(out=outp[sti * P : (sti + 1) * P, :], in_=ob[:])

    def _attn(hp, g, interleave=None):
        kjs = list(range(4 * g, NST))  # ascending
        pairs = [(kjs[2 * i], kjs[2 * i + 1]) for i in range(len(kjs) // 2)]
        pv = pv_pool.tile([P, 1024], F32, tag="pv", name="pv")
        zeroed = []

        def zero_banks():
            # pre-zero both banks so packed slots accumulate with start=False.
            # Deferred past the first scores so the WAR on the previous g's
            # normalize doesn't block the PE head.
            nc.tensor.matmul(
                pv[:, 0:455], lhsT=zeros_sb[:, 0:P], rhs=zeros_sb[:, 0:455],
                start=True, stop=True, skip_group_check=True,
            )
            nc.tensor.matmul(
                pv[:, 512 : 512 + 65], lhsT=zeros_sb[:, 0:P], rhs=zeros_sb[:, 0:65],
                start=True, stop=True, skip_group_check=True,
            )
            zeroed.append(True)

        def emit_pv(units):
            if not zeroed:
                zero_banks()
            for pt, kj, coff in units:
                for h in range(2):
                    hc = 2 * hp + h
                    for ql in range(min(kj - 4 * g, 3) + 1):
                        nc.tensor.matmul(
                            pv_slot(pv, h, ql),
                            lhsT=pt[h][:, coff + P * ql : coff + P * (ql + 1)],
                            rhs=v_sb[kj][:, hc, :],
                            start=False,
                            stop=(kj == NST - 1),
                            skip_group_check=True,
                        )

        prev = None
        for pi, (kj0, kj1) in enumerate(pairs):
            if interleave is not None:
                interleave(2 * pi)
            diag = kj1 - 4 * g <= 3
            wid = {
                kj: (P * (kj - 4 * g + 1) if kj - 4 * g <= 3 else QCH)
                for kj in (kj0, kj1)
            }
            # PSUM start=True zeroes a whole 2KB bank, and matmul outputs
            # must not straddle banks: wider chunk first at offset 0
            # (start=True); the narrower one either starts at the bank-1
            # boundary (start=True) or overwrites pending-zeroed bytes of
            # bank 0 (start=False).
            if diag:
                parts = [(kj1, 0, True), (kj0, wid[kj1], wid[kj1] == QCH)]
            else:
                parts = [(kj0, 0, True), (kj1, QCH, True)]
            tot = wid[kj0] + wid[kj1]
            pts = []
            for h in range(2):
                st = st_pool.tile([P, 1024], F32, tag="st", name="st")
                b32 = slice(32 * h, 32 * h + 32)
                for kj, coff, sflag in parts:
                    nc.tensor.matmul(
                        st[:, coff : coff + wid[kj]],
                        lhsT=k8_sb[hp][b32, :, kj * P : (kj + 1) * P],
                        rhs=q8_sb[hp][b32, :, g * QCH : g * QCH + wid[kj]],
                        start=sflag,
                        stop=True,
                        skip_group_check=not sflag,
                        perf_mode=DR,
                    )
                if prev is not None and h == 1:
                    emit_pv(prev)
                    prev = None
                pt = pt_pool.tile([P, 1024], F16, tag="pt", name="pt")
                nc.scalar.activation(
                    pt[:, 0:tot], st[:, 0:tot], EXP, scale=float(ESCALE)
                )
                if diag:
                    for kj, coff, _ in parts:
                        tri = slice(coff + wid[kj] - P, coff + wid[kj])
                        nc.gpsimd.affine_select(
                            out=pt[:, tri],
                            in_=pt[:, tri],
                            compare_op=mybir.AluOpType.is_ge,
                            fill=0.0,
                            base=0,
                            channel_multiplier=1,
                            pattern=[[-1, P]],
                        )
                pts.append(pt)
                if interleave is not None and h == 0:
                    interleave(2 * pi + 1)
            prev = [(pts, kj, coff) for kj, coff, _ in parts]
        emit_pv(prev)

        # normalize: per (head, q-subchunk): 1/rowsum (ones column) then
        # per-partition scale into attn_sb [q, (h, d)]
        for h in range(2):
            for ql in range(4):
                qq = 4 * g + ql
                sl = pv_slot(pv, h, ql)
                nc.vector.reciprocal(
                    out=rinv_sb[hp][:, qq, h : h + 1], in_=sl[:, HD : HD + 1]
                )
                nc.vector.tensor_scalar_mul(
                    attn_sb[hp][:, qq, HD * h : HD * (h + 1)],
                    sl[:, 0:HD],
                    rinv_sb[hp][:, qq, h : h + 1],
                )

    # --- phase 1: projections + hp0 attention ------------------------------
    with tc.tile_pool(name="proj_psum", bufs=2, space="PSUM") as pp:

        def qk_proj(proj, j, schs):
            dst = qt_sb if proj == 0 else kt_sb
            dst8 = q8_sb if proj == 0 else k8_sb
            woff = proj * DSL + j * P
            for sch in schs:
                ssl = slice(sch * QCH, (sch + 1) * QCH)
                ps = pp.tile([P, QCH], F32, tag="pp", name="pp")
                for c in range(4):
                    nc.tensor.matmul(
                        ps[:],
                        lhsT=w8_t[:, c, :, woff : woff + P],
                        rhs=x8_t[:, c, :, sch * QCH : (sch + 1) * QCH],
                        start=(c == 0),
                        stop=(c == 3),
                        perf_mode=DR,
                    )
                # bias add (per-partition, prescaled by 32) + fp8 cast
                nc.vector.tensor_scalar_add(dst[j][:, ssl], ps[:], bias_sb[:, j, proj : proj + 1])
                # partition remap into the DoubleRow score layout. The host
                # permutes Q/K weight columns to (i, hl, r) order, so psum
                # partition p = 64*i + 32*hl + r maps to dst8[32*hl + r, i, s]
                # with one contiguous 64-partition DMA per contraction half.
                # Plain 2D APs only (partition-split rearranges break dep
                # tracking). j=0 chunks are needed progressively (per sch);
                # j=1 in full-row DMAs once all four chunks are evicted.
                if j == 0:
                    for i in range(2):
                        nc.sync.dma_start(
                            out=dst8[j][0:64, i, ssl],
                            in_=dst[j][64 * i : 64 * i + 64, ssl],
                        )
                elif sch == 3:
                    for i in range(2):
                        nc.sync.dma_start(
                            out=dst8[j][0:64, i, :],
                            in_=dst[j][64 * i : 64 * i + 64, :],
                        )

        def v_proj(st):
            ps = pp.tile([P, DSL], F32, tag="pp", name="ppv")
            ssl = slice(st * P, (st + 1) * P)
            vw = slice(2 * DSL, 3 * DSL)
            k = 0
            for lhs_t, rhs_t, rsl in (
                (x8_t, w8_t, vw),
                (x8_t, wvr8_t, slice(0, DSL)),
                (xr8_t, w8_t, vw),
            ):
                for c in range(4):
                    nc.tensor.matmul(
                        ps[:],
                        lhsT=lhs_t[:, c, :, ssl],
                        rhs=rhs_t[:, c, :, rsl],
                        start=(k == 0),
                        stop=(k == 11),
                        perf_mode=DR,
                    )
                    k += 1
            # descale (1/32) + fp16 cast
            nc.vector.tensor_scalar_mul(
                v_sb[st][:, :, 0:HD],
                ps[:].rearrange("p (h d) -> p h d", h=HPC),
                1.0 / WS,
            )
            nc.vector.memset(v_sb[st][:, :, HD : HD + 1], 1.0)

        qk_proj(0, 0, [0])
        qk_proj(1, 0, [0])
        early_loads2()

        # units interleaved into attn(0,0): v tiles ascending + remaining
        # j0 projection chunks in first-use order
        a00_units = {
            2: [("k", 0, 1)],
            4: [("q", 0, 1)],
            6: [("k", 0, 2)],
            8: [("q", 0, 2)],
            10: [("k", 0, 3)],
            12: [("q", 0, 3)],
        }

        def emit_a00(hi):
            if hi == 1:
                late_loads(2)
            elif hi == 5:
                late_loads(3)
            elif hi == 9:
                late_loads(4)
            if hi >= 2:
                v_proj(hi - 2)
            if hi == NST - 1:
                v_proj(NST - 2)
                v_proj(NST - 1)
            for kind, j, s in a00_units.get(hi, ()):
                qk_proj(0 if kind == "q" else 1, j, [s])

        _attn(0, 0, interleave=emit_a00)

        j1_units = [("q", 1, s) for s in range(NQC)] + [
            ("k", 1, s) for s in range(NQC)
        ]

        def emit_j1(hi):
            if hi % 2 == 0 and j1_units:
                kind, j, s = j1_units.pop(0)
                qk_proj(0 if kind == "q" else 1, j, [s])

        _attn(0, 1, interleave=emit_j1)
        _attn(0, 2, interleave=emit_j1)
        _attn(0, 3)

    # projection pool closed: 2 PSUM banks free for outproj/transposes
    op_psum_cell.append(
        ctx.enter_context(tc.tile_pool(name="op_psum", bufs=2, space="PSUM"))
    )

    # interleave units for the hp1 phase: hp0 transposes during g=0, then
    # hp1 transposes + output projections as their inputs become ready.
    def make_units(g):
        units = []
        if g == 0:
            for qq in range(NST):
                units.append(("t", 0, qq))
        else:
            for qq in range(4 * (g - 1), 4 * g):
                units.append(("t", 1, qq))
            for sti in range(4 * (g - 1), 4 * g):
                units.append(("o", sti, 0))
                units.append(("o", sti, 1))
        return units

    for g in range(NQC):
        units = make_units(g)
        npts = NST - 4 * g  # interleave points (2 per pair)
        per = (len(units) + npts - 1) // npts

        def emit_units(hi, units=units, per=per):
            for _ in range(per):
                if units:
                    kind, a, b = units.pop(0)
                    if kind == "t":
                        _transp(a, b)
                    else:
                        outproj_unit(a, b)

        _attn(1, g, interleave=emit_units)
        while units:
            kind, a, b = units.pop(0)
            if kind == "t":
                _transp(a, b)
            else:
                outproj_unit(a, b)

    # tail: last transposes + output projections
    for qq in range(12, NST):
        _transp(1, qq)
    for sti, kind in ((12, "st"), (13, "op"), (14, "pv"), (15, "st")):
        outproj_unit(sti, 0, slotkind=kind)
        outproj_unit(sti, 1, slotkind=kind)

    ctx.close()


_NC_CACHE = None


def _get_nc():
    global _NC_CACHE
    if _NC_CACHE is None:
        nc = bacc.Bacc("TRN2", target_bir_lowering=False, debug=False)
        _NC_CACHE = _build_kernel(nc)
    return _NC_CACHE


def _pairs(a, cols):
    """[1024, cols] fp8 -> [128, 4, 2, cols] pair-interleaved layout:
    out[p, c, i, s] = a[256c + 128i + p, s]."""
    return np.ascontiguousarray(
        np.asarray(a, dtype=NP8).reshape(4, 2, P, cols).transpose(2, 0, 1, 3)
    )


def kernel(x, Wq, bq, Wk, bk, Wv, bv, Wo, bo):
    x = np.asarray(x, dtype=np.float32)
    Wq, bq = np.asarray(Wq, np.float32), np.asarray(bq, np.float32)
    Wk, bk = np.asarray(Wk, np.float32), np.asarray(bk, np.float32)
    Wv, bv = np.asarray(Wv, np.float32), np.asarray(bv, np.float32)
    Wo, bo = np.asarray(Wo, np.float32), np.asarray(bo, np.float32)

    nc = _get_nc()

    ident = np.eye(P, dtype=np.float16)
    in_maps = []
    for c in range(N_CORES):
        b = c // 4
        hg = c % 4
        hsl = slice(hg * DSL, (hg + 1) * DSL)
        xT = np.ascontiguousarray(x[b].T)  # [1024, 2048]
        x8 = np.asarray(xT, dtype=NP8)
        xr = xT - x8.astype(np.float32)
        # Q/K columns permuted per 128-block to (i, hl, r) order so the
        # fp8-score remap DMAs are contiguous; V stays in (h, d) order.
        perm = (
            np.arange(256).reshape(2, 2, 2, 32).transpose(0, 2, 1, 3).reshape(256)
        )  # (j, hl, i, r) -> (j, i, hl, r)
        wqkv = np.concatenate(
            [WS * Wq[hsl].T[:, perm], WS * Wk[hsl].T[:, perm], WS * Wv[hsl].T],
            axis=1,
        )  # [1024, 768]
        w8 = np.asarray(wqkv, dtype=NP8)
        wvr = wqkv[:, 2 * DSL :] - w8[:, 2 * DSL :].astype(np.float32)
        bqk = np.stack(
            [WS * bq[hsl][perm], WS * bk[hsl][perm]], axis=1
        ).reshape(2, P, 2)
        bqk = np.ascontiguousarray(bqk.transpose(1, 0, 2), dtype=np.float32)
        in_maps.append(
            {
                "x8p": _pairs(x8, S),
                "xr8p": _pairs(xr, S),
                "w8p": _pairs(w8, 3 * DSL),
                "wvr8p": _pairs(wvr, DSL),
                "woT": np.ascontiguousarray(
                    Wo[:, hsl].T.reshape(2, P, D), dtype=np.float16
                ),
                "bqk": bqk,
                "ident": ident,
            }
        )

    res = run_bass_kernel_spmd(
        nc, in_maps, core_ids=list(range(N_CORES)), trace=False
    )

    # host gather: sum partials per batch, add analytic bias terms
    bias_term = (bv @ Wo.T + bo).astype(np.float32)  # (D,)
    out = np.empty((B, S, D), dtype=np.float32)
    for b in range(B):
        acc = res.results[4 * b]["outp"].astype(np.float32)
        for c in range(4 * b + 1, 4 * b + 4):
            acc = acc + res.results[c]["outp"].astype(np.float32)
        out[b] = acc + bias_term
    return out


# revision 18
# speedup vs baseline: 1.0774x; 1.0774x over previous
"""Multi-head attention (B=2, S=2048, D=1024, H=16, Hd=64) on 8 trn2 cores.

Sharding: batch x head-group. Core c handles batch c//4 and heads
[4*(c%4), 4*(c%4)+4).

Numerics (validated in numpy against the fp32 reference, rel ~4e-3):
- Q/K projections: fp8e4m3 x and fp8 weights (pre-scaled by 32 to avoid
  the fp8 subnormal region), DoubleRow perf mode: each matmul contracts
  256 rows at 0.5 cycles/output-column.
- V projection: compensated fp8 (x8*Wv8 + x8*rv8 + r8*Wv8 where r8/rv8
  are fp8 residuals) - plain fp8 V fails the 2e-2 gate, fp16 costs 2x.
- Scores: fp16 Q/K operands (64-dim contraction per head).
- Softmax: exp on ScalarE with scale 1/(32*32*32) (weight prescale
  folded in); no max-subtraction needed (|score| < ~0.4).
- PV: flipped layout - P^T tiles become the stationary operand so the
  output lands as [query-partitions, head-dim]; the ones-column of V
  gives the softmax row-sum per query partition, so normalization is a
  per-partition reciprocal + tensor_scalar multiply (no DRAM bounce).
- Attention outputs are transposed back to [dl, q] with PE transposes
  (identity matmul) for the fp16 output projection; partial outputs are
  stored fp16 and summed on the host (bv @ Wo.T + bo added analytically).

Mask: reference keeps the *upper* triangle (key >= query): query q
attends keys k >= q; tiles strictly below the block diagonal are skipped.
Key-chunk pairs are processed ascending so V tiles and KT chunks are
consumed in DMA-arrival order; diagonal pairs pack the second key
chunk's scores adjacent to the first so one exp covers both.

PSUM budget (8 banks): scores 2x[128,1024] (4) + PV accumulators
[128,1024] x1 (2) + outproj/transpose shared pool (2). PV packs 8
(head, q-chunk) accumulators of 65 fp32 into 2 banks; a cheap
zero-outer-product matmul pre-zeroes each bank so the packed slots can
all accumulate with start=False (PSUM start=True zeroing is
2KB-region-granular).
"""

import contextlib

import os as _os

_jp = _os.environ.get("JAX_PLATFORMS", "")
if _jp and "axon" not in _jp:
    _os.environ["JAX_PLATFORMS"] = "axon," + _jp

import numpy as np
import ml_dtypes

import concourse.bass as bass
import concourse.tile as tile
from concourse import bacc, mybir
from concourse.bass_utils import run_bass_kernel_spmd

F32 = mybir.dt.float32
F16 = mybir.dt.float16
F8 = mybir.dt.float8e4
NP8 = ml_dtypes.float8_e4m3

B = 2
S = 2048
D = 1024
HD = 64
N_CORES = 8
HPC = 4  # heads per core
DSL = HPC * HD  # 256 projection columns per core
P = 128
NST = S // P  # 16 seq tiles
QCH = 512
NQC = S // QCH  # 4

WS = 32.0  # weight prescale (keeps fp8 weights out of the subnormal range)
ESCALE = 1.0 / (np.sqrt(np.float32(D)) * WS * WS)  # exp scale: 1/32768

DR = mybir.MatmulPerfMode.DoubleRow
EXP = mybir.ActivationFunctionType.Exp


def _build_kernel(nc: bass.Bass, repeat: int = 1):
    x8d = nc.dram_tensor("x8p", (P, 4, 2, S), F8, kind="ExternalInput").ap()
    xr8d = nc.dram_tensor("xr8p", (P, 4, 2, S), F8, kind="ExternalInput").ap()
    w8d = nc.dram_tensor("w8p", (P, 4, 2, 3 * DSL), F8, kind="ExternalInput").ap()
    wvr8d = nc.dram_tensor("wvr8p", (P, 4, 2, DSL), F8, kind="ExternalInput").ap()
    wod = nc.dram_tensor("woT", (2, P, D), F16, kind="ExternalInput").ap()
    bqkd = nc.dram_tensor("bqk", (P, 2, 2), F32, kind="ExternalInput").ap()
    identd = nc.dram_tensor("ident", (P, P), F16, kind="ExternalInput").ap()
    outp = nc.dram_tensor("outp", (S, D), F16, kind="ExternalOutput").ap()

    with tile.TileContext(nc) as tc:
        for _ in range(repeat):
            _emit(tc, nc, x8d, xr8d, w8d, wvr8d, wod, bqkd, identd, outp)
    nc.compile()
    return nc


def _emit(tc, nc, x8d, xr8d, w8d, wvr8d, wod, bqkd, identd, outp):
    ctx = contextlib.ExitStack()

    persist = ctx.enter_context(tc.tile_pool(name="persist", bufs=1))

    x8_t = persist.tile([P, 4, 2, S], F8, tag="x8", name="x8")
    xr8_t = persist.tile([P, 4, 2, S], F8, tag="xr8", name="xr8")
    w8_t = persist.tile([P, 4, 2, 3 * DSL], F8, tag="w8", name="w8")
    wvr8_t = persist.tile([P, 4, 2, DSL], F8, tag="wvr", name="wvr")
    wo_t = [persist.tile([P, D], F16, tag=f"wo{j}", name=f"wo{j}") for j in range(2)]
    bias_sb = persist.tile([P, 2, 2], F32, tag="bias", name="bias")
    ident_sb = persist.tile([P, P], F16, tag="ident", name="ident")
    zeros_sb = persist.tile([P, QCH], F16, tag="zeros", name="zeros")
    qt_sb = [persist.tile([P, S], F8, tag=f"qt{j}", name=f"qt{j}") for j in range(2)]
    kt_sb = [persist.tile([P, S], F8, tag=f"kt{j}", name=f"kt{j}") for j in range(2)]
    q8_sb = [persist.tile([P, 2, S], F8, tag=f"q8{j}", name=f"q8{j}") for j in range(2)]
    k8_sb = [persist.tile([P, 2, S], F8, tag=f"k8{j}", name=f"k8{j}") for j in range(2)]
    v_sb = [
        persist.tile([P, HPC, HD + 1], F16, tag=f"v{i}", name=f"v{i}")
        for i in range(NST)
    ]
    attn_sb = [
        persist.tile([P, NST, P], F16, tag=f"attn{j}", name=f"attn{j}")
        for j in range(2)
    ]
    attnt_sb = [
        persist.tile([P, S], F16, tag=f"attnt{j}", name=f"attnt{j}") for j in range(2)
    ]
    rinv_sb = [
        persist.tile([P, NST, 2], F32, tag=f"rinv{j}", name=f"rinv{j}")
        for j in range(2)
    ]

    nc.vector.memset(zeros_sb[:], 0.0)

    # --- input DMAs (SP queue; s-quartered so early seq chunks arrive first;
    # Q/K weight columns before V so the first score tile is unblocked ASAP)
    nc.sync.dma_start(out=bias_sb[:], in_=bqkd)
    nc.sync.dma_start(out=x8_t[:, :, :, 0:QCH], in_=x8d[:, :, :, 0:QCH])
    nc.sync.dma_start(out=w8_t[:, :, :, 0 : 2 * DSL], in_=w8d[:, :, :, 0 : 2 * DSL])
    nc.sync.dma_start(out=xr8_t[:, :, :, 0:QCH], in_=xr8d[:, :, :, 0:QCH])

    def early_loads2():
        nc.sync.dma_start(
            out=x8_t[:, :, :, QCH : 2 * QCH], in_=x8d[:, :, :, QCH : 2 * QCH]
        )
        nc.sync.dma_start(
            out=w8_t[:, :, :, 2 * DSL : 3 * DSL], in_=w8d[:, :, :, 2 * DSL : 3 * DSL]
        )
        nc.sync.dma_start(out=wvr8_t[:], in_=wvr8d)
        nc.sync.dma_start(
            out=xr8_t[:, :, :, QCH : 2 * QCH], in_=xr8d[:, :, :, QCH : 2 * QCH]
        )
        nc.sync.dma_start(out=ident_sb[:], in_=identd)

    def late_loads(q4):
        if q4 < 4:
            ssl = slice(q4 * QCH, (q4 + 1) * QCH)
            nc.sync.dma_start(out=x8_t[:, :, :, ssl], in_=x8d[:, :, :, ssl])
            nc.sync.dma_start(out=xr8_t[:, :, :, ssl], in_=xr8d[:, :, :, ssl])
        else:
            for j in range(2):
                nc.sync.dma_start(out=wo_t[j][:], in_=wod[j])

    st_pool = ctx.enter_context(tc.tile_pool(name="st_psum", bufs=2, space="PSUM"))
    pv_pool = ctx.enter_context(tc.tile_pool(name="pv_psum", bufs=1, space="PSUM"))
    pt_pool = ctx.enter_context(tc.tile_pool(name="pt", bufs=6))
    nsc_pool = ctx.enter_context(tc.tile_pool(name="nsc", bufs=2))
    ob_pool = ctx.enter_context(tc.tile_pool(name="ob", bufs=4))
    op_psum_cell = []

    # PV accumulator slot: m = 2*ql + h; slots 0-6 packed in bank 0,
    # slot 7 at the start of bank 1 (matmul outputs may not straddle banks).
    def pv_slot(pv, h, ql):
        m = 2 * ql + h
        if m < 7:
            return pv[:, 65 * m : 65 * m + 65]
        return pv[:, 512 : 512 + 65]

    def _transp(hp, qq):
        tp = op_psum_cell[0].tile([P, P], F16, tag="op", name="tp")
        nc.tensor.transpose(tp[:], attn_sb[hp][:, qq, :], ident_sb[:])
        if hp == 1 and qq >= 12:
            nc.scalar.copy(attnt_sb[hp][:, qq * P : (qq + 1) * P], tp[:])
        else:
            nc.vector.tensor_copy(attnt_sb[hp][:, qq * P : (qq + 1) * P], tp[:])

    ob_tiles = {}

    def outproj_unit(sti, e, slotkind="op"):
        if e == 0:
            ob_tiles[sti] = ob_pool.tile([P, D], F16, tag="ob", name="ob")
        ob = ob_tiles[sti]
        if slotkind == "st":
            # scores are done: borrow a free st-pool bank for extra slots
            stt = st_pool.tile([P, 1024], F32, tag="st", name="sttail")
            op = stt[:, e * QCH : (e + 1) * QCH]
        elif slotkind == "pv":
            pvt = pv_pool.tile([P, 1024], F32, tag="pv", name="pvtail")
            op = pvt[:, e * QCH : (e + 1) * QCH]
        else:
            op = op_psum_cell[0].tile([P, QCH], F32, tag="op", name="op")[:]
        for j in range(2):
            nc.tensor.matmul(
                op,
                lhsT=attnt_sb[j][:, sti * P : (sti + 1) * P],
                rhs=wo_t[j][:, e * QCH : (e + 1) * QCH],
                start=(j == 0),
                stop=(j == 1),
                skip_group_check=(slotkind == "pv"),
            )
        esl = slice(e * QCH, (e + 1) * QCH)
        # GPSIMD cannot read PSUM: evictions go to DVE; late ones to the
        # mostly-idle ScalarE
        if sti >= 8 and e == 0:
            nc.scalar.copy(ob[:, esl], op)
        else:
            nc.vector.tensor_copy(ob[:, esl], op)
        if e == 1:
            nc.sync.dma_start(out=outp[sti * P : (sti + 1) * P, :], in_=ob[:])

    def _attn(hp, g, interleave=None):
        kjs = list(range(4 * g, NST))  # ascending
        pairs = [(kjs[2 * i], kjs[2 * i + 1]) for i in range(len(kjs) // 2)]
        pv = pv_pool.tile([P, 1024], F32, tag="pv", name="pv")
        zeroed = []

        def zero_banks():
            # pre-zero both banks so packed slots accumulate with start=False.
            # Deferred past the first scores so the WAR on the previous g's
            # normalize doesn't block the PE head.
            nc.tensor.matmul(
                pv[:, 0:455], lhsT=zeros_sb[:, 0:P], rhs=zeros_sb[:, 0:455],
                start=True, stop=True, skip_group_check=True,
            )
            nc.tensor.matmul(
                pv[:, 512 : 512 + 65], lhsT=zeros_sb[:, 0:P], rhs=zeros_sb[:, 0:65],
                start=True, stop=True, skip_group_check=True,
            )
            zeroed.append(True)

        def emit_pv(units):
            if not zeroed:
                zero_banks()
            for pt, kj, coff in units:
                for h in range(2):
                    hc = 2 * hp + h
                    for ql in range(min(kj - 4 * g, 3) + 1):
                        nc.tensor.matmul(
                            pv_slot(pv, h, ql),
                            lhsT=pt[h][:, coff + P * ql : coff + P * (ql + 1)],
                            rhs=v_sb[kj][:, hc, :],
                            start=False,
                            stop=(kj == NST - 1),
                            skip_group_check=True,
                        )

        prev = None
        for pi, (kj0, kj1) in enumerate(pairs):
            if interleave is not None:
                interleave(2 * pi)
            diag = kj1 - 4 * g <= 3
            wid = {
                kj: (P * (kj - 4 * g + 1) if kj - 4 * g <= 3 else QCH)
                for kj in (kj0, kj1)
            }
            # PSUM start=True zeroes a whole 2KB bank, and matmul outputs
            # must not straddle banks: wider chunk first at offset 0
            # (start=True); the narrower one either starts at the bank-1
            # boundary (start=True) or overwrites pending-zeroed bytes of
            # bank 0 (start=False).
            if diag:
                parts = [(kj1, 0, True), (kj0, wid[kj1], wid[kj1] == QCH)]
            else:
                parts = [(kj0, 0, True), (kj1, QCH, True)]
            tot = wid[kj0] + wid[kj1]
            pts = []
            for h in range(2):
                st = st_pool.tile([P, 1024], F32, tag="st", name="st")
                b32 = slice(32 * h, 32 * h + 32)
                for kj, coff, sflag in parts:
                    nc.tensor.matmul(
                        st[:, coff : coff + wid[kj]],
                        lhsT=k8_sb[hp][b32, :, kj * P : (kj + 1) * P],
                        rhs=q8_sb[hp][b32, :, g * QCH : g * QCH + wid[kj]],
                        start=sflag,
                        stop=True,
                        skip_group_check=not sflag,
                        perf_mode=DR,
                    )
                if prev is not None and h == 1:
                    emit_pv(prev)
                    prev = None
                pt = pt_pool.tile([P, 1024], F16, tag="pt", name="pt")
                nc.scalar.activation(
                    pt[:, 0:tot], st[:, 0:tot], EXP, scale=float(ESCALE)
                )
                if diag:
                    for kj, coff, _ in parts:
                        tri = slice(coff + wid[kj] - P, coff + wid[kj])
                        nc.gpsimd.affine_select(
                            out=pt[:, tri],
                            in_=pt[:, tri],
                            compare_op=mybir.AluOpType.is_ge,
                            fill=0.0,
                            base=0,
                            channel_multiplier=1,
                            pattern=[[-1, P]],
                        )
                pts.append(pt)
                if interleave is not None and h == 0:
                    interleave(2 * pi + 1)
            prev = [(pts, kj, coff) for kj, coff, _ in parts]
        emit_pv(prev)

        # normalize: copy the 8 packed accumulators to SBUF scratch (the
        # next g's pre-zero then only WARs these 2 copies, not 16 slow
        # normalize ops), take all 8 reciprocals in one strided op, then
        # scale per (head, q-subchunk) into attn_sb [q, (h, d)]
        nsc = nsc_pool.tile([P, 520], F32, tag="nsc", name="nsc")
        nc.vector.tensor_copy(nsc[:, 0:455], pv[:, 0:455])
        nc.vector.tensor_copy(nsc[:, 455:520], pv[:, 512 : 512 + 65])
        rr = rinv_sb[hp][:, 4 * g : 4 * g + 4, :]
        nc.vector.reciprocal(
            out=rr,
            in_=bass.AP(
                tensor=nsc.tensor,
                offset=nsc.offset + HD,
                ap=[list(nsc.ap[0]), [65, 8]],
            ),
        )
        for h in range(2):
            for ql in range(4):
                qq = 4 * g + ql
                m = 2 * ql + h
                nc.vector.tensor_scalar_mul(
                    attn_sb[hp][:, qq, HD * h : HD * (h + 1)],
                    nsc[:, 65 * m : 65 * m + 64],
                    rinv_sb[hp][:, qq, h : h + 1],
                )

    # --- phase 1: projections + hp0 attention ------------------------------
    with tc.tile_pool(name="proj_psum", bufs=2, space="PSUM") as pp:

        def qk_proj(proj, j, schs):
            dst = qt_sb if proj == 0 else kt_sb
            dst8 = q8_sb if proj == 0 else k8_sb
            woff = proj * DSL + j * P
            for sch in schs:
                ssl = slice(sch * QCH, (sch + 1) * QCH)
                ps = pp.tile([P, QCH], F32, tag="pp", name="pp")
                for c in range(4):
                    nc.tensor.matmul(
                        ps[:],
                        lhsT=w8_t[:, c, :, woff : woff + P],
                        rhs=x8_t[:, c, :, sch * QCH : (sch + 1) * QCH],
                        start=(c == 0),
                        stop=(c == 3),
                        perf_mode=DR,
                    )
                # bias add (per-partition, prescaled by 32) + fp8 cast
                nc.vector.tensor_scalar_add(dst[j][:, ssl], ps[:], bias_sb[:, j, proj : proj + 1])
                # partition remap into the DoubleRow score layout. The host
                # permutes Q/K weight columns to (i, hl, r) order, so psum
                # partition p = 64*i + 32*hl + r maps to dst8[32*hl + r, i, s]
                # with one contiguous 64-partition DMA per contraction half.
                # Plain 2D APs only (partition-split rearranges break dep
                # tracking). j=0 chunks are needed progressively (per sch);
                # j=1 in full-row DMAs once all four chunks are evicted.
                if j == 0:
                    for i in range(2):
                        nc.sync.dma_start(
                            out=dst8[j][0:64, i, ssl],
                            in_=dst[j][64 * i : 64 * i + 64, ssl],
                        )
                elif sch == 3:
                    for i in range(2):
                        nc.sync.dma_start(
                            out=dst8[j][0:64, i, :],
                            in_=dst[j][64 * i : 64 * i + 64, :],
                        )

        def v_proj(st):
            ps = pp.tile([P, DSL], F32, tag="pp", name="ppv")
            ssl = slice(st * P, (st + 1) * P)
            vw = slice(2 * DSL, 3 * DSL)
            k = 0
            for lhs_t, rhs_t, rsl in (
                (x8_t, w8_t, vw),
                (x8_t, wvr8_t, slice(0, DSL)),
                (xr8_t, w8_t, vw),
            ):
                for c in range(4):
                    nc.tensor.matmul(
                        ps[:],
                        lhsT=lhs_t[:, c, :, ssl],
                        rhs=rhs_t[:, c, :, rsl],
                        start=(k == 0),
                        stop=(k == 11),
                        perf_mode=DR,
                    )
                    k += 1
            # descale (1/32) + fp16 cast
            nc.vector.tensor_scalar_mul(
                v_sb[st][:, :, 0:HD],
                ps[:].rearrange("p (h d) -> p h d", h=HPC),
                1.0 / WS,
            )
            nc.vector.memset(v_sb[st][:, :, HD : HD + 1], 1.0)

        qk_proj(0, 0, [0])
        qk_proj(1, 0, [0])
        early_loads2()

        # units interleaved into attn(0,0): v tiles ascending + remaining
        # j0 projection chunks in first-use order
        a00_units = {
            1: [("k", 0, 1)],
            4: [("q", 0, 1)],
            6: [("k", 0, 2)],
            8: [("q", 0, 2)],
            10: [("k", 0, 3)],
            12: [("q", 0, 3)],
        }

        def emit_a00(hi):
            if hi == 1:
                late_loads(2)
            elif hi == 5:
                late_loads(3)
            elif hi == 9:
                late_loads(4)
            if hi >= 2:
                v_proj(hi - 2)
            if hi == NST - 1:
                v_proj(NST - 2)
                v_proj(NST - 1)
            for kind, j, s in a00_units.get(hi, ()):
                qk_proj(0 if kind == "q" else 1, j, [s])

        _attn(0, 0, interleave=emit_a00)

        j1_units = [("q", 1, s) for s in range(NQC)] + [
            ("k", 1, s) for s in range(NQC)
        ]

        def emit_j1(hi):
            if hi % 2 == 0 and j1_units:
                kind, j, s = j1_units.pop(0)
                qk_proj(0 if kind == "q" else 1, j, [s])

        _attn(0, 1, interleave=emit_j1)
        _attn(0, 2, interleave=emit_j1)
        _attn(0, 3)

    # projection pool closed: 2 PSUM banks free for outproj/transposes
    op_psum_cell.append(
        ctx.enter_context(tc.tile_pool(name="op_psum", bufs=2, space="PSUM"))
    )

    # interleave units for the hp1 phase: hp0 transposes during g=0, then
    # hp1 transposes + output projections as their inputs become ready.
    def make_units(g):
        units = []
        if g == 0:
            for qq in range(NST):
                units.append(("t", 0, qq))
        else:
            for qq in range(4 * (g - 1), 4 * g):
                units.append(("t", 1, qq))
            for sti in range(4 * (g - 1), 4 * g):
                units.append(("o", sti, 0))
                units.append(("o", sti, 1))
        return units

    for g in range(NQC):
        units = make_units(g)
        npts = NST - 4 * g  # interleave points (2 per pair)
        per = (len(units) + npts - 1) // npts

        def emit_units(hi, units=units, per=per):
            for _ in range(per):
                if units:
                    kind, a, b = units.pop(0)
                    if kind == "t":
                        _transp(a, b)
                    else:
                        outproj_unit(a, b)

        _attn(1, g, interleave=emit_units)
        while units:
            kind, a, b = units.pop(0)
            if kind == "t":
                _transp(a, b)
            else:
                outproj_unit(a, b)

    # tail: last transposes + output projections
    for qq in range(12, NST):
        _transp(1, qq)
    for sti, kind in ((12, "st"), (13, "op"), (14, "pv"), (15, "st")):
        outproj_unit(sti, 0, slotkind=kind)
        outproj_unit(sti, 1, slotkind=kind)

    ctx.close()


_NC_CACHE = None


def _get_nc():
    global _NC_CACHE
    if _NC_CACHE is None:
        nc = bacc.Bacc("TRN2", target_bir_lowering=False, debug=False)
        _NC_CACHE = _build_kernel(nc)
    return _NC_CACHE


def _pairs(a, cols):
    """[1024, cols] fp8 -> [128, 4, 2, cols] pair-interleaved layout:
    out[p, c, i, s] = a[256c + 128i + p, s]."""
    return np.ascontiguousarray(
        np.asarray(a, dtype=NP8).reshape(4, 2, P, cols).transpose(2, 0, 1, 3)
    )


def kernel(x, Wq, bq, Wk, bk, Wv, bv, Wo, bo):
    x = np.asarray(x, dtype=np.float32)
    Wq, bq = np.asarray(Wq, np.float32), np.asarray(bq, np.float32)
    Wk, bk = np.asarray(Wk, np.float32), np.asarray(bk, np.float32)
    Wv, bv = np.asarray(Wv, np.float32), np.asarray(bv, np.float32)
    Wo, bo = np.asarray(Wo, np.float32), np.asarray(bo, np.float32)

    nc = _get_nc()

    ident = np.eye(P, dtype=np.float16)
    in_maps = []
    for c in range(N_CORES):
        b = c // 4
        hg = c % 4
        hsl = slice(hg * DSL, (hg + 1) * DSL)
        xT = np.ascontiguousarray(x[b].T)  # [1024, 2048]
        x8 = np.asarray(xT, dtype=NP8)
        xr = xT - x8.astype(np.float32)
        # Q/K columns permuted per 128-block to (i, hl, r) order so the
        # fp8-score remap DMAs are contiguous; V stays in (h, d) order.
        perm = (
            np.arange(256).reshape(2, 2, 2, 32).transpose(0, 2, 1, 3).reshape(256)
        )  # (j, hl, i, r) -> (j, i, hl, r)
        wqkv = np.concatenate(
            [WS * Wq[hsl].T[:, perm], WS * Wk[hsl].T[:, perm], WS * Wv[hsl].T],
            axis=1,
        )  # [1024, 768]
        w8 = np.asarray(wqkv, dtype=NP8)
        wvr = wqkv[:, 2 * DSL :] - w8[:, 2 * DSL :].astype(np.float32)
        bqk = np.stack(
            [WS * bq[hsl][perm], WS * bk[hsl][perm]], axis=1
        ).reshape(2, P, 2)
        bqk = np.ascontiguousarray(bqk.transpose(1, 0, 2), dtype=np.float32)
        in_maps.append(
            {
                "x8p": _pairs(x8, S),
                "xr8p": _pairs(xr, S),
                "w8p": _pairs(w8, 3 * DSL),
                "wvr8p": _pairs(wvr, DSL),
                "woT": np.ascontiguousarray(
                    Wo[:, hsl].T.reshape(2, P, D), dtype=np.float16
                ),
                "bqk": bqk,
                "ident": ident,
            }
        )

    res = run_bass_kernel_spmd(
        nc, in_maps, core_ids=list(range(N_CORES)), trace=False
    )

    # host gather: sum partials per batch, add analytic bias terms
    bias_term = (bv @ Wo.T + bo).astype(np.float32)  # (D,)
    out = np.empty((B, S, D), dtype=np.float32)
    for b in range(B):
        acc = res.results[4 * b]["outp"].astype(np.float32)
        for c in range(4 * b + 1, 4 * b + 4):
            acc = acc + res.results[c]["outp"].astype(np.float32)
        out[b] = acc + bias_term
    return out


# revision 25
# speedup vs baseline: 1.0801x; 1.0025x over previous
"""Multi-head attention (B=2, S=2048, D=1024, H=16, Hd=64) on 8 trn2 cores.

Sharding: batch x head-group. Core c handles batch c//4 and heads
[4*(c%4), 4*(c%4)+4).

Numerics (validated in numpy against the fp32 reference, rel ~4e-3):
- Q/K projections: fp8e4m3 x and fp8 weights (pre-scaled by 32 to avoid
  the fp8 subnormal region), DoubleRow perf mode: each matmul contracts
  256 rows at 0.5 cycles/output-column.
- V projection: compensated fp8 (x8*Wv8 + x8*rv8 + r8*Wv8 where r8/rv8
  are fp8 residuals) - plain fp8 V fails the 2e-2 gate, fp16 costs 2x.
- Scores: fp8 Q/K operands in DoubleRow mode - each head's 64-dim
  contraction is split into two 32-partition halves interleaved along
  the free dim (a per-sch DMA remap of the projection output, made
  contiguous by a host-side weight-column permutation).
- Softmax: exp on ScalarE with scale 1/(32*32*32) (weight prescale
  folded in); no max-subtraction needed (|score| < ~0.4).
- PV: flipped layout - P^T tiles become the stationary operand so the
  output lands as [query-partitions, head-dim]; the ones-column of V
  gives the softmax row-sum per query partition, so normalization is a
  per-partition reciprocal + tensor_scalar multiply (no DRAM bounce).
- Attention outputs are transposed back to [dl, q] with PE transposes
  (identity matmul) for the fp16 output projection; partial outputs are
  stored fp16 and summed on the host (bv @ Wo.T + bo added analytically).

Mask: reference keeps the *upper* triangle (key >= query): query q
attends keys k >= q; tiles strictly below the block diagonal are skipped.
Key-chunk pairs are processed ascending so V tiles and KT chunks are
consumed in DMA-arrival order; diagonal pairs pack the second key
chunk's scores adjacent to the first so one exp covers both.

PSUM budget (8 banks): scores 2x[128,1024] (4) + PV accumulators
[128,1024] x1 (2) + outproj/transpose shared pool (2). PV packs 8
(head, q-chunk) accumulators of 65 fp32 into 2 banks; a cheap
zero-outer-product matmul pre-zeroes each bank so the packed slots can
all accumulate with start=False (PSUM start=True zeroing is
2KB-region-granular).
"""

import contextlib

import os as _os

_jp = _os.environ.get("JAX_PLATFORMS", "")
if _jp and "axon" not in _jp:
    _os.environ["JAX_PLATFORMS"] = "axon," + _jp

import numpy as np
import ml_dtypes

import concourse.bass as bass
import concourse.tile as tile
from concourse import bacc, mybir
from concourse.bass_utils import run_bass_kernel_spmd

F32 = mybir.dt.float32
F16 = mybir.dt.float16
F8 = mybir.dt.float8e4
NP8 = ml_dtypes.float8_e4m3

B = 2
S = 2048
D = 1024
HD = 64
N_CORES = 8
HPC = 4  # heads per core
DSL = HPC * HD  # 256 projection columns per core
P = 128
NST = S // P  # 16 seq tiles
QCH = 512
NQC = S // QCH  # 4

WS = 32.0  # weight prescale (keeps fp8 weights out of the subnormal range)
ESCALE = 1.0 / (np.sqrt(np.float32(D)) * WS * WS)  # exp scale: 1/32768

DR = mybir.MatmulPerfMode.DoubleRow
EXP = mybir.ActivationFunctionType.Exp


def _build_kernel(nc: bass.Bass, repeat: int = 1):
    x8d = nc.dram_tensor("x8p", (P, 4, 2, S), F8, kind="ExternalInput").ap()
    xr8d = nc.dram_tensor("xr8p", (P, 4, 2, S), F8, kind="ExternalInput").ap()
    w8d = nc.dram_tensor("w8p", (P, 4, 2, 3 * DSL), F8, kind="ExternalInput").ap()
    wvr8d = nc.dram_tensor("wvr8p", (P, 4, 2, DSL), F8, kind="ExternalInput").ap()
    wod = nc.dram_tensor("woT", (2, P, D), F16, kind="ExternalInput").ap()
    bqkd = nc.dram_tensor("bqk", (P, 2, 2), F32, kind="ExternalInput").ap()
    identd = nc.dram_tensor("ident", (P, P), F16, kind="ExternalInput").ap()
    outp = nc.dram_tensor("outp", (S, D), F16, kind="ExternalOutput").ap()

    with tile.TileContext(nc) as tc:
        for _ in range(repeat):
            _emit(tc, nc, x8d, xr8d, w8d, wvr8d, wod, bqkd, identd, outp)
    nc.compile()
    return nc


def _emit(tc, nc, x8d, xr8d, w8d, wvr8d, wod, bqkd, identd, outp):
    ctx = contextlib.ExitStack()

    persist = ctx.enter_context(tc.tile_pool(name="persist", bufs=1))

    x8_t = persist.tile([P, 4, 2, S], F8, tag="x8", name="x8")
    xr8_t = persist.tile([P, 4, 2, S], F8, tag="xr8", name="xr8")
    w8_t = persist.tile([P, 4, 2, 3 * DSL], F8, tag="w8", name="w8")
    wvr8_t = persist.tile([P, 4, 2, DSL], F8, tag="wvr", name="wvr")
    wo_t = [persist.tile([P, D], F16, tag=f"wo{j}", name=f"wo{j}") for j in range(2)]
    bias_sb = persist.tile([P, 2, 2], F32, tag="bias", name="bias")
    ident_sb = persist.tile([P, P], F16, tag="ident", name="ident")
    zeros_sb = persist.tile([P, QCH], F16, tag="zeros", name="zeros")
    qt_sb = [persist.tile([P, S], F8, tag=f"qt{j}", name=f"qt{j}") for j in range(2)]
    kt_sb = [persist.tile([P, S], F8, tag=f"kt{j}", name=f"kt{j}") for j in range(2)]
    q8_sb = [persist.tile([P, 2, S], F8, tag=f"q8{j}", name=f"q8{j}") for j in range(2)]
    k8_sb = [persist.tile([P, 2, S], F8, tag=f"k8{j}", name=f"k8{j}") for j in range(2)]
    v_sb = [
        persist.tile([P, HPC, HD + 1], F16, tag=f"v{i}", name=f"v{i}")
        for i in range(NST)
    ]
    attn_sb = [
        persist.tile([P, NST, P], F16, tag=f"attn{j}", name=f"attn{j}")
        for j in range(2)
    ]
    attnt_sb = [
        persist.tile([P, S], F16, tag=f"attnt{j}", name=f"attnt{j}") for j in range(2)
    ]
    rinv_sb = [
        persist.tile([P, NST, 2], F32, tag=f"rinv{j}", name=f"rinv{j}")
        for j in range(2)
    ]

    nc.vector.memset(zeros_sb[:], 0.0)

    # --- input DMAs (SP queue; s-quartered so early seq chunks arrive first;
    # Q/K weight columns before V so the first score tile is unblocked ASAP)
    QS = [slice(q4 * QCH, (q4 + 1) * QCH) for q4 in range(4)]
    nc.sync.dma_start(out=bias_sb[:], in_=bqkd)
    nc.sync.dma_start(out=w8_t[:, :, :, 0:DSL], in_=w8d[:, :, :, 0:DSL])
    nc.sync.dma_start(out=x8_t[:, :, :, QS[3]], in_=x8d[:, :, :, QS[3]])
    nc.sync.dma_start(out=w8_t[:, :, :, DSL : 2 * DSL], in_=w8d[:, :, :, DSL : 2 * DSL])

    def early_loads2():
        nc.sync.dma_start(out=xr8_t[:, :, :, QS[3]], in_=xr8d[:, :, :, QS[3]])
        nc.sync.dma_start(
            out=w8_t[:, :, :, 2 * DSL : 3 * DSL], in_=w8d[:, :, :, 2 * DSL : 3 * DSL]
        )
        nc.sync.dma_start(out=wvr8_t[:], in_=wvr8d)
        nc.sync.dma_start(out=x8_t[:, :, :, QS[2]], in_=x8d[:, :, :, QS[2]])
        nc.sync.dma_start(out=xr8_t[:, :, :, QS[2]], in_=xr8d[:, :, :, QS[2]])
        nc.sync.dma_start(out=x8_t[:, :, :, QS[1]], in_=x8d[:, :, :, QS[1]])
        nc.sync.dma_start(out=ident_sb[:], in_=identd)

    def late_loads(q4):
        if q4 >= 0:
            if q4 != 1:
                nc.sync.dma_start(out=x8_t[:, :, :, QS[q4]], in_=x8d[:, :, :, QS[q4]])
            nc.sync.dma_start(out=xr8_t[:, :, :, QS[q4]], in_=xr8d[:, :, :, QS[q4]])
        else:
            for j in range(2):
                nc.sync.dma_start(out=wo_t[j][:], in_=wod[j])

    st_pool = ctx.enter_context(tc.tile_pool(name="st_psum", bufs=2, space="PSUM"))
    pv_pool = ctx.enter_context(tc.tile_pool(name="pv_psum", bufs=1, space="PSUM"))
    pt_pool = ctx.enter_context(tc.tile_pool(name="pt", bufs=6))
    nsc_pool = ctx.enter_context(tc.tile_pool(name="nsc", bufs=2))
    ob_pool = ctx.enter_context(tc.tile_pool(name="ob", bufs=4))
    op_psum_cell = []

    # PV accumulator slot: m = 2*ql + h; slots 0-6 packed in bank 0,
    # slot 7 at the start of bank 1 (matmul outputs may not straddle banks).
    def pv_slot(pv, h, ql):
        m = 2 * ql + h
        if m < 7:
            return pv[:, 65 * m : 65 * m + 65]
        return pv[:, 512 : 512 + 65]

    def _transp(hp, qq):
        tp = op_psum_cell[0].tile([P, P], F16, tag="op", name="tp")
        nc.tensor.transpose(tp[:], attn_sb[hp][:, qq, :], ident_sb[:])
        if hp == 1 and qq >= 12:
            nc.scalar.copy(attnt_sb[hp][:, qq * P : (qq + 1) * P], tp[:])
        else:
            nc.vector.tensor_copy(attnt_sb[hp][:, qq * P : (qq + 1) * P], tp[:])

    ob_tiles = {}

    def outproj_unit(sti, e, slotkind="op"):
        if e == 0:
            ob_tiles[sti] = ob_pool.tile([P, D], F16, tag="ob", name="ob")
        ob = ob_tiles[sti]
        if slotkind == "st":
            # scores are done: borrow a free st-pool bank for extra slots
            stt = st_pool.tile([P, 1024], F32, tag="st", name="sttail")
            op = stt[:, e * QCH : (e + 1) * QCH]
        elif slotkind == "pv":
            pvt = pv_pool.tile([P, 1024], F32, tag="pv", name="pvtail")
            op = pvt[:, e * QCH : (e + 1) * QCH]
        else:
            op = op_psum_cell[0].tile([P, QCH], F32, tag="op", name="op")[:]
        for j in range(2):
            nc.tensor.matmul(
                op,
                lhsT=attnt_sb[j][:, sti * P : (sti + 1) * P],
                rhs=wo_t[j][:, e * QCH : (e + 1) * QCH],
                start=(j == 0),
                stop=(j == 1),
                skip_group_check=(slotkind == "pv"),
            )
        esl = slice(e * QCH, (e + 1) * QCH)
        # GPSIMD cannot read PSUM: evictions go to DVE; late ones to the
        # mostly-idle ScalarE
        if sti >= 8 and e == 0:
            nc.scalar.copy(ob[:, esl], op)
        else:
            nc.vector.tensor_copy(ob[:, esl], op)
        if e == 1:
            nc.sync.dma_start(out=outp[sti * P : (sti + 1) * P, :], in_=ob[:])

    def _attn(hp, g, interleave=None):
        kjs = list(range(4 * g, NST))  # ascending
        pairs = [(kjs[2 * i], kjs[2 * i + 1]) for i in range(len(kjs) // 2)]
        pv = pv_pool.tile([P, 1024], F32, tag="pv", name="pv")
        zeroed = []

        def zero_banks():
            # pre-zero both banks so packed slots accumulate with start=False.
            # Deferred past the first scores so the WAR on the previous g's
            # normalize doesn't block the PE head.
            nc.tensor.matmul(
                pv[:, 0:455], lhsT=zeros_sb[:, 0:P], rhs=zeros_sb[:, 0:455],
                start=True, stop=True, skip_group_check=True,
            )
            nc.tensor.matmul(
                pv[:, 512 : 512 + 65], lhsT=zeros_sb[:, 0:P], rhs=zeros_sb[:, 0:65],
                start=True, stop=True, skip_group_check=True,
            )
            zeroed.append(True)

        def emit_pv(units):
            if not zeroed:
                zero_banks()
            for pt, kj, coff in units:
                for h in range(2):
                    hc = 2 * hp + h
                    for ql in range(min(kj - 4 * g, 3) + 1):
                        nc.tensor.matmul(
                            pv_slot(pv, h, ql),
                            lhsT=pt[h][:, coff + P * ql : coff + P * (ql + 1)],
                            rhs=v_sb[kj][:, hc, :],
                            start=False,
                            stop=(kj == NST - 1),
                            skip_group_check=True,
                        )

        prev = None
        for pi, (kj0, kj1) in enumerate(pairs):
            if interleave is not None:
                interleave(2 * pi)
            diag = kj1 - 4 * g <= 3
            wid = {
                kj: (P * (kj - 4 * g + 1) if kj - 4 * g <= 3 else QCH)
                for kj in (kj0, kj1)
            }
            # PSUM start=True zeroes a whole 2KB bank, and matmul outputs
            # must not straddle banks: wider chunk first at offset 0
            # (start=True); the narrower one either starts at the bank-1
            # boundary (start=True) or overwrites pending-zeroed bytes of
            # bank 0 (start=False).
            if diag:
                parts = [(kj1, 0, True), (kj0, wid[kj1], wid[kj1] == QCH)]
            else:
                parts = [(kj0, 0, True), (kj1, QCH, True)]
            tot = wid[kj0] + wid[kj1]
            pts = []
            for h in range(2):
                st = st_pool.tile([P, 1024], F32, tag="st", name="st")
                b32 = slice(32 * h, 32 * h + 32)
                for kj, coff, sflag in parts:
                    nc.tensor.matmul(
                        st[:, coff : coff + wid[kj]],
                        lhsT=k8_sb[hp][b32, :, kj * P : (kj + 1) * P],
                        rhs=q8_sb[hp][b32, :, g * QCH : g * QCH + wid[kj]],
                        start=sflag,
                        stop=True,
                        skip_group_check=not sflag,
                        perf_mode=DR,
                    )
                if prev is not None and h == 1:
                    emit_pv(prev)
                    prev = None
                pt = pt_pool.tile([P, 1024], F16, tag="pt", name="pt")
                nc.scalar.activation(
                    pt[:, 0:tot], st[:, 0:tot], EXP, scale=float(ESCALE)
                )
                if diag:
                    for kj, coff, _ in parts:
                        tri = slice(coff + wid[kj] - P, coff + wid[kj])
                        nc.gpsimd.affine_select(
                            out=pt[:, tri],
                            in_=pt[:, tri],
                            compare_op=mybir.AluOpType.is_ge,
                            fill=0.0,
                            base=0,
                            channel_multiplier=1,
                            pattern=[[-1, P]],
                        )
                pts.append(pt)
                if interleave is not None and h == 0:
                    interleave(2 * pi + 1)
            prev = [(pts, kj, coff) for kj, coff, _ in parts]
        emit_pv(prev)

        # normalize: copy the 8 packed accumulators to SBUF scratch (the
        # next g's pre-zero then only WARs these 2 copies, not 16 slow
        # normalize ops), take all 8 reciprocals in one strided op, then
        # scale per (head, q-subchunk) into attn_sb [q, (h, d)]
        nsc = nsc_pool.tile([P, 520], F32, tag="nsc", name="nsc")
        nc.vector.tensor_copy(nsc[:, 0:455], pv[:, 0:455])
        nc.vector.tensor_copy(nsc[:, 455:520], pv[:, 512 : 512 + 65])
        rr = rinv_sb[hp][:, 4 * g : 4 * g + 4, :]
        nc.vector.reciprocal(
            out=rr,
            in_=bass.AP(
                tensor=nsc.tensor,
                offset=nsc.offset + HD,
                ap=[list(nsc.ap[0]), [65, 8]],
            ),
        )
        for h in range(2):
            for ql in range(4):
                qq = 4 * g + ql
                m = 2 * ql + h
                nc.vector.tensor_scalar_mul(
                    attn_sb[hp][:, qq, HD * h : HD * (h + 1)],
                    nsc[:, 65 * m : 65 * m + 64],
                    rinv_sb[hp][:, qq, h : h + 1],
                )

    # --- phase 1: projections + hp0 attention ------------------------------
    with tc.tile_pool(name="proj_psum", bufs=2, space="PSUM") as pp:

        def qk_proj(proj, j, schs):
            dst = qt_sb if proj == 0 else kt_sb
            dst8 = q8_sb if proj == 0 else k8_sb
            woff = proj * DSL + j * P
            for sch in schs:
                ssl = slice(sch * QCH, (sch + 1) * QCH)
                ps = pp.tile([P, QCH], F32, tag="pp", name="pp")
                for c in range(4):
                    nc.tensor.matmul(
                        ps[:],
                        lhsT=w8_t[:, c, :, woff : woff + P],
                        rhs=x8_t[:, c, :, sch * QCH : (sch + 1) * QCH],
                        start=(c == 0),
                        stop=(c == 3),
                        perf_mode=DR,
                    )
                # bias add (per-partition, prescaled by 32) + fp8 cast
                nc.vector.tensor_scalar_add(dst[j][:, ssl], ps[:], bias_sb[:, j, proj : proj + 1])
                # partition remap into the DoubleRow score layout. The host
                # permutes Q/K weight columns to (i, hl, r) order, so psum
                # partition p = 64*i + 32*hl + r maps to dst8[32*hl + r, i, s]
                # with one contiguous 64-partition DMA per contraction half.
                # Plain 2D APs only (partition-split rearranges break dep
                # tracking). j=0 chunks are needed progressively (per sch);
                # j=1 in full-row DMAs once all four chunks are evicted.
                for i in range(2):
                    nc.sync.dma_start(
                        out=dst8[j][0:64, i, ssl],
                        in_=dst[j][64 * i : 64 * i + 64, ssl],
                    )

        def v_proj(st):
            ps = pp.tile([P, DSL], F32, tag="pp", name="ppv")
            ssl = slice(st * P, (st + 1) * P)
            vw = slice(2 * DSL, 3 * DSL)
            k = 0
            for lhs_t, rhs_t, rsl in (
                (x8_t, w8_t, vw),
                (x8_t, wvr8_t, slice(0, DSL)),
                (xr8_t, w8_t, vw),
            ):
                for c in range(4):
                    nc.tensor.matmul(
                        ps[:],
                        lhsT=lhs_t[:, c, :, ssl],
                        rhs=rhs_t[:, c, :, rsl],
                        start=(k == 0),
                        stop=(k == 11),
                        perf_mode=DR,
                    )
                    k += 1
            # descale (1/32) + fp16 cast
            nc.vector.tensor_scalar_mul(
                v_sb[st][:, :, 0:HD],
                ps[:].rearrange("p (h d) -> p h d", h=HPC),
                1.0 / WS,
            )
            nc.vector.memset(v_sb[st][:, :, HD : HD + 1], 1.0)

        qk_proj(0, 0, [3])
        qk_proj(1, 0, [3])
        early_loads2()
        qk_proj(0, 0, [2])
        qk_proj(1, 0, [2])

        # hp0 runs g descending so V projections (and their x/w inputs)
        # spread evenly: attn(0,g) only needs the 4 new v tiles 4g..4g+3
        # for its first two (diagonal) pairs; later kjs reuse earlier ones.
        j1_units = [("q", 1, 0), ("k", 1, 0), ("k", 1, 1), ("q", 1, 1),
                    ("k", 1, 2), ("q", 1, 2), ("k", 1, 3), ("q", 1, 3)]
        g_units = {
            3: {2: [("v", 12), ("v", 13)], 3: [("v", 14), ("v", 15), ("x", 1)]},
            2: {2: [("v", 8), ("v", 9)], 3: [("k", 0, 1)],
                4: [("v", 10), ("v", 11)], 5: [("q", 0, 1)], 7: [("x", 0)]},
            1: {2: [("v", 4), ("v", 5)], 3: [("k", 0, 0)],
                4: [("v", 6), ("v", 7)], 5: [("q", 0, 0)], 7: [("x", -1)],
                9: [j1_units[0]], 11: [j1_units[1]]},
            0: {2: [("v", 0), ("v", 1)], 3: [j1_units[2]],
                4: [("v", 2), ("v", 3)], 5: [j1_units[3]],
                6: [j1_units[4]], 8: [j1_units[5]],
                10: [j1_units[6]], 12: [j1_units[7]]},
        }

        def make_emit(g):
            units = g_units[g]

            def emit(hi):
                for u in units.get(hi, ()):
                    if u[0] == "v":
                        v_proj(u[1])
                    elif u[0] == "x":
                        late_loads(u[1])
                    else:
                        qk_proj(0 if u[0] == "q" else 1, u[1], [u[2]])

            return emit

        for g in (3, 2, 1, 0):
            _attn(0, g, interleave=make_emit(g))

    # projection pool closed: 2 PSUM banks free for outproj/transposes
    op_psum_cell.append(
        ctx.enter_context(tc.tile_pool(name="op_psum", bufs=2, space="PSUM"))
    )

    # interleave units for the hp1 phase: hp0 transposes during g=0, then
    # hp1 transposes + output projections as their inputs become ready.
    def make_units(g):
        units = []
        if g == 0:
            for qq in range(NST):
                units.append(("t", 0, qq))
        else:
            for qq in range(4 * (g - 1), 4 * g):
                units.append(("t", 1, qq))
            for sti in range(4 * (g - 1), 4 * g):
                units.append(("o", sti, 0))
                units.append(("o", sti, 1))
        return units

    for g in range(NQC):
        units = make_units(g)
        npts = NST - 4 * g  # interleave points (2 per pair)
        per = (len(units) + npts - 1) // npts

        def emit_units(hi, units=units, per=per):
            for _ in range(per):
                if units:
                    kind, a, b = units.pop(0)
                    if kind == "t":
                        _transp(a, b)
                    else:
                        outproj_unit(a, b)

        _attn(1, g, interleave=emit_units)
        while units:
            kind, a, b = units.pop(0)
            if kind == "t":
                _transp(a, b)
            else:
                outproj_unit(a, b)

    # tail: last transposes + output projections
    for qq in range(12, NST):
        _transp(1, qq)
    for sti, kind in ((12, "st"), (13, "op"), (14, "pv"), (15, "st")):
        outproj_unit(sti, 0, slotkind=kind)
        outproj_unit(sti, 1, slotkind=kind)

    ctx.close()


_NC_CACHE = None


def _get_nc():
    global _NC_CACHE
    if _NC_CACHE is None:
        nc = bacc.Bacc("TRN2", target_bir_lowering=False, debug=False)
        _NC_CACHE = _build_kernel(nc)
    return _NC_CACHE


def _pairs(a, cols):
    """[1024, cols] fp8 -> [128, 4, 2, cols] pair-interleaved layout:
    out[p, c, i, s] = a[256c + 128i + p, s]."""
    return np.ascontiguousarray(
        np.asarray(a, dtype=NP8).reshape(4, 2, P, cols).transpose(2, 0, 1, 3)
    )


def kernel(x, Wq, bq, Wk, bk, Wv, bv, Wo, bo):
    x = np.asarray(x, dtype=np.float32)
    Wq, bq = np.asarray(Wq, np.float32), np.asarray(bq, np.float32)
    Wk, bk = np.asarray(Wk, np.float32), np.asarray(bk, np.float32)
    Wv, bv = np.asarray(Wv, np.float32), np.asarray(bv, np.float32)
    Wo, bo = np.asarray(Wo, np.float32), np.asarray(bo, np.float32)

    nc = _get_nc()

    ident = np.eye(P, dtype=np.float16)
    in_maps = []
    for c in range(N_CORES):
        b = c // 4
        hg = c % 4
        hsl = slice(hg * DSL, (hg + 1) * DSL)
        xT = np.ascontiguousarray(x[b].T)  # [1024, 2048]
        x8 = np.asarray(xT, dtype=NP8)
        xr = xT - x8.astype(np.float32)
        # Q/K columns permuted per 128-block to (i, hl, r) order so the
        # fp8-score remap DMAs are contiguous; V stays in (h, d) order.
        perm = (
            np.arange(256).reshape(2, 2, 2, 32).transpose(0, 2, 1, 3).reshape(256)
        )  # (j, hl, i, r) -> (j, i, hl, r)
        wqkv = np.concatenate(
            [WS * Wq[hsl].T[:, perm], WS * Wk[hsl].T[:, perm], WS * Wv[hsl].T],
            axis=1,
        )  # [1024, 768]
        w8 = np.asarray(wqkv, dtype=NP8)
        wvr = wqkv[:, 2 * DSL :] - w8[:, 2 * DSL :].astype(np.float32)
        bqk = np.stack(
            [WS * bq[hsl][perm], WS * bk[hsl][perm]], axis=1
        ).reshape(2, P, 2)
        bqk = np.ascontiguousarray(bqk.transpose(1, 0, 2), dtype=np.float32)
        in_maps.append(
            {
                "x8p": _pairs(x8, S),
                "xr8p": _pairs(xr, S),
                "w8p": _pairs(w8, 3 * DSL),
                "wvr8p": _pairs(wvr, DSL),
                "woT": np.ascontiguousarray(
                    Wo[:, hsl].T.reshape(2, P, D), dtype=np.float16
                ),
                "bqk": bqk,
                "ident": ident,
            }
        )

    res = run_bass_kernel_spmd(
        nc, in_maps, core_ids=list(range(N_CORES)), trace=False
    )

    # host gather: sum partials per batch, add analytic bias terms
    bias_term = (bv @ Wo.T + bo).astype(np.float32)  # (D,)
    out = np.empty((B, S, D), dtype=np.float32)
    for b in range(B):
        acc = res.results[4 * b]["outp"].astype(np.float32)
        for c in range(4 * b + 1, 4 * b + 4):
            acc = acc + res.results[c]["outp"].astype(np.float32)
        out[b] = acc + bias_term
    return out
